# revision 1
# baseline (speedup 1.0000x reference)
"""Trainium2 Bass kernel for nn_Network_63763084476816 (GNN message passing).

The batched graph is structurally fixed: per graph, 38 clinical + 36 pixel
nodes, self-edges everywhere, and a complete bipartite pixel<->clinical edge
set.  Mean aggregation therefore collapses to dense math:

    h_c = relu(x_c @ (W_self + W_msg/37) + S_pix @ (W_msg/37) + b_g)
    h_p = relu(x_p @ (W_self + W_msg/39) + S_clin @ (W_msg/39) + b_g)
    gap = mean_p h_p
    out = relu([h_c | gap] @ W1 + b1) @ W2 + b2

Sharding: pure data parallel, 128 graphs per core on 8 cores; weights
(including the 10 MB W1) replicated.  Embeddings ship in a feature-major,
node-major layout ([FV, node*BC + b]) so every matmul operand already has
its contraction dim on partitions - no on-chip transposes.

Matmuls run in float32r (single-pass fp32 on the PE, 4x the throughput of
two-pass float32; N kept >= 256 everywhere so the fast path applies).  The
h phase processes 4 node blocks per PSUM bank with one N=512 matmul pair:
x-part with A stationary, then the per-graph aggregate term with W_msg/deg
stationary against a 4x-replicated S tile.  b1 is added with a K=1 matmul
into the same accumulation group; the final [512]->1 layer runs as three
plain DVE ops.  Node sums use contiguous tree-folds plus one short strided
reduce instead of a fully strided reduction.
"""

import sys

for _p in ("/opt/trn_rl_repo",):
    if _p not in sys.path:
        sys.path.insert(0, _p)

import numpy as np

_B = 1024
_NCORES = 8
_BC = _B // _NCORES  # 128 graphs per core
_NCLIN = 38
_NPIX = 36
_FV = 128
_HID = 512
_NCHUNK = 39  # K-chunks of 128 in the 4992-wide MLP contraction
# K-chunks per W1 DMA group; last group tiny so the MLP tail after the
# final W1 arrival is one matmul.
_W1GROUPS = [8, 8, 8, 8, 6, 1]
_CCOLS = _NCLIN * _BC  # 4864
_PCOLS = _NPIX * _BC  # 4608

_CACHE = {}


def _build_bass():
    import concourse.bacc as bacc
    import concourse.mybir as mybir
    import concourse.tile as tile

    f32 = mybir.dt.float32
    f32r = mybir.dt.float32r
    relu = mybir.ActivationFunctionType.Relu
    ax = mybir.AxisListType.X

    nc = bacc.Bacc("TRN2", target_bir_lowering=False, debug=False,
                   num_devices=_NCORES)

    xt_d = nc.dram_tensor("xt", [_FV, _CCOLS + _PCOLS], f32r, kind="ExternalInput")
    # W1 arrives host-packed in the SBUF layout: [p, (chunk, n)] — every DMA
    # reads long per-partition contiguous runs.
    w1_d = nc.dram_tensor("w1", [_FV, _NCHUNK * _HID], f32r, kind="ExternalInput")
    gw_d = nc.dram_tensor("gw", [_FV, 4 * _FV], f32r, kind="ExternalInput")
    aux_d = nc.dram_tensor("aux", [_BC, _HID + 3], f32, kind="ExternalInput")
    rowaux_d = nc.dram_tensor("rowaux", [1, _HID + _BC], f32r, kind="ExternalInput")
    out_d = nc.dram_tensor("out", [_BC, 1], f32, kind="ExternalOutput")

    with tile.TileContext(nc) as tc:
        with tc.tile_pool(name="main", bufs=1) as pool, \
             tc.tile_pool(name="hps", bufs=6, space="PSUM") as pps, \
             tc.tile_pool(name="zps", bufs=1, space="PSUM") as ppz:

            # Small parameter loads on the scalar (ACT) HWDGE ring so they
            # don't delay the big streams on the sync (SP) ring.
            gwsb = pool.tile([_FV, 4 * _FV], f32r, name="gwsb", tag="gwsb")
            nc.scalar.dma_start(gwsb[:], gw_d.ap())
            auxsb = pool.tile([_BC, _HID + 3], f32, name="auxsb", tag="auxsb")
            nc.scalar.dma_start(auxsb[:], aux_d.ap())
            rowsb = pool.tile([1, _HID + _BC], f32r, name="rowsb", tag="rowsb")
            nc.scalar.dma_start(rowsb[:], rowaux_d.ap())

            # Node embeddings, feature-major.  Pixel section first (its sum
            # gates the clinical h blocks, which run first), in two halves so
            # the S_pix partial sums start before the full section lands.
            xt = pool.tile([_FV, _CCOLS + _PCOLS], f32r, name="xt", tag="xt")
            _PH = _PCOLS // 2  # 2304 = 18 pixel blocks
            nc.sync.dma_start(xt[:, _CCOLS:_CCOLS + _PH], xt_d.ap()[:, _CCOLS:_CCOLS + _PH])
            nc.sync.dma_start(xt[:, _CCOLS + _PH:], xt_d.ap()[:, _CCOLS + _PH:])
            nc.sync.dma_start(xt[:, :_CCOLS], xt_d.ap()[:, :_CCOLS])

            # W1 streamed in 5 groups; group g holds K-chunks as [FV, gch, HID].
            # W1 after xt on the same sync ring: FIFO order doubles as a
            # priority order, so the xt stream (which gates all compute)
            # never contends with the W1 stream.
            w1sb = []
            c0 = 0
            for g, gch in enumerate(_W1GROUPS):
                t = pool.tile([_FV, gch, _HID], f32r, name=f"w1sb{g}", tag=f"w1sb{g}")
                nc.sync.dma_start(
                    t[:],
                    w1_d.ap()[:, c0 * _HID:(c0 + gch) * _HID].rearrange(
                        "p (c n) -> p c n", c=gch),
                )
                w1sb.append(t)
                c0 += gch

            # Per-graph node sums S[f, b], replicated to 4 copies for the
            # N=512 aggregate matmuls.  Contiguous tree-folds first, then a
            # short strided reduce over the remaining blocks.
            u = pool.tile([_FV, 2432], f32, name="u", tag="u")
            v = pool.tile([_FV, 1216], f32, name="v", tag="v")

            _LOWP = "float32r matmul operands; accumulation stays fp32"

            # S_pix from per-half partial sums: each 18-block half folds to 9
            # blocks then a short strided reduce; halves land independently.
            s4pix = pool.tile([_FV, 4 * _BC], f32r, name="s4pix", tag="s4pix")
            sh1 = pool.tile([_FV, _BC], f32, name="sh1", tag="sh1")
            sh2 = pool.tile([_FV, _BC], f32, name="sh2", tag="sh2")
            nc.vector.tensor_add(u[:, :1152], xt[:, _CCOLS:_CCOLS + 1152],
                                 xt[:, _CCOLS + 1152:_CCOLS + 2304])
            nc.vector.reduce_sum(
                sh1[:], u[:, :1152].rearrange("f (p b) -> f b p", p=9), axis=ax)
            nc.vector.tensor_add(v[:, :1152], xt[:, _CCOLS + 2304:_CCOLS + 3456],
                                 xt[:, _CCOLS + 3456:])
            nc.vector.reduce_sum(
                sh2[:], v[:, :1152].rearrange("f (p b) -> f b p", p=9), axis=ax)
            with nc.allow_low_precision(reason=_LOWP):
                nc.vector.tensor_add(s4pix[:, :_BC], sh1[:], sh2[:])
            nc.vector.tensor_copy(s4pix[:, _BC:2 * _BC], s4pix[:, :_BC])
            nc.vector.tensor_copy(s4pix[:, 2 * _BC:], s4pix[:, :2 * _BC])

            # S_clin: one fold to 19 blocks, then two shorter strided reduces.
            s4clin = pool.tile([_FV, 4 * _BC], f32r, name="s4clin", tag="s4clin")
            nc.vector.tensor_add(u[:, :2432], xt[:, :2432], xt[:, 2432:_CCOLS])
            nc.vector.reduce_sum(
                sh1[:], u[:, :1152].rearrange("f (c b) -> f b c", c=9), axis=ax)
            nc.vector.reduce_sum(
                sh2[:], u[:, 1152:2432].rearrange("f (c b) -> f b c", c=10), axis=ax)
            with nc.allow_low_precision(reason=_LOWP):
                nc.vector.tensor_add(s4clin[:, :_BC], sh1[:], sh2[:])
            nc.vector.tensor_copy(s4clin[:, _BC:2 * _BC], s4clin[:, :_BC])
            nc.vector.tensor_copy(s4clin[:, 2 * _BC:], s4clin[:, :2 * _BC])

            combT = pool.tile([_FV, _NCHUNK * _BC], f32r, name="combT", tag="combT")
            hpT = pool.tile([_FV, _PCOLS], f32r, name="hpT", tag="hpT")
            bg_ap = auxsb[:, _HID:_HID + 1]

            def h_phase(nblk, a_ap, wm_ap, s4_ap, src0, dest, psname):
                g0, gi = 0, 0
                while g0 < nblk:
                    gcnt = min(4, nblk - g0)
                    w = gcnt * _BC
                    ps = pps.tile([_FV, w], f32, name=f"{psname}{gi}", tag="hps")
                    nc.tensor.matmul(
                        ps[:], a_ap,
                        xt[:, src0 + g0 * _BC: src0 + (g0 + gcnt) * _BC],
                        start=True, stop=False,
                    )
                    nc.tensor.matmul(
                        ps[:], wm_ap, s4_ap[:, :w],
                        start=False, stop=True,
                    )
                    nc.scalar.activation(
                        dest[:, g0 * _BC: g0 * _BC + w], ps[:], relu, bias=bg_ap,
                    )
                    g0 += gcnt
                    gi += 1

            # h^T tiles: clinical into combT blocks 0..37, pixel into hpT.
            h_phase(_NCLIN, gwsb[:, 0:_FV], gwsb[:, 2 * _FV:3 * _FV], s4pix,
                    0, combT, "psc")
            h_phase(_NPIX, gwsb[:, _FV:2 * _FV], gwsb[:, 3 * _FV:4 * _FV], s4clin,
                    _CCOLS, hpT, "psp")

            # gap block (plain sum; the 1/36 is folded into W1's last rows).
            nc.vector.tensor_add(u[:, :2304], hpT[:, :2304], hpT[:, 2304:])
            nc.vector.tensor_add(v[:, :1152], u[:, :1152], u[:, 1152:2304])
            with nc.allow_low_precision(reason=_LOWP):
                nc.vector.reduce_sum(
                    combT[:, _NCLIN * _BC:],
                    v[:, :1152].rearrange("f (p b) -> f b p", p=9), axis=ax)

            # MLP layer 1: psz[b, n] = sum_k combined[b, k] W1[k, n] (+ b1).
            # Emission order = PE FIFO order: early-arriving W1 groups first,
            # then the b1 matmul and the gap chunk (ready mid-stream), and the
            # last-arriving W1 groups at the end so nothing head-blocks.
            psz = ppz.tile([_BC, _HID], f32, name="psz", tag="psz")

            def mlp_chunk(k, start, stop):
                goff = 0
                for g, gch in enumerate(_W1GROUPS):
                    if k < goff + gch:
                        nc.tensor.matmul(
                            psz[:],
                            combT[:, k * _BC:(k + 1) * _BC],
                            w1sb[g][:, k - goff, :],
                            start=start, stop=stop,
                        )
                        return
                    goff += gch

            for k in range(32):  # groups 0-3 (chunks 0..31)
                mlp_chunk(k, start=(k == 0), stop=False)
            nc.tensor.matmul(psz[:], rowsb[:, _HID:], rowsb[:, :_HID],
                             start=False, stop=False)  # + b1
            for k in range(32, 38):  # group 4
                mlp_chunk(k, start=False, stop=False)
            # chunk 38 = gap x W1 group 5: both the gap h-values and the last
            # W1 bytes are the latest to arrive, so this goes last.
            mlp_chunk(38, start=False, stop=True)

            # MLP layer 2 fused: one DVE op does relu (max with 0), the W2
            # multiply, and the free-dim sum, reading psz directly from PSUM.
            # (tensor_tensor_reduce wedges the device on this path;
            # scalar_tensor_tensor with accum_out is HW-verified.)
            zw = pool.tile([_BC, _HID], f32, name="zw", tag="zw")
            osum = pool.tile([_BC, 1], f32, name="osum", tag="osum")
            nc.vector.scalar_tensor_tensor(
                out=zw[:], in0=psz[:], scalar=0.0, in1=auxsb[:, :_HID],
                op0=mybir.AluOpType.max, op1=mybir.AluOpType.mult,
                accum_out=osum[:],
            )
            ofin = pool.tile([_BC, 1], f32, name="ofin", tag="ofin")
            nc.vector.tensor_add(ofin[:], osum[:], auxsb[:, _HID + 1:_HID + 2])
            nc.sync.dma_start(out_d.ap(), ofin[:])

    nc.compile()
    return nc


def _host_prep(W_self, W_msg, b_g, W1, b1, W2, b2):
    f32 = np.float32
    wmc = np.asarray(W_msg, f32) / f32(37.0)
    wmp = np.asarray(W_msg, f32) / f32(39.0)
    ws = np.asarray(W_self, f32)
    gw = np.ascontiguousarray(
        np.hstack([ws + wmc, ws + wmp, wmc, wmp]).astype(f32))
    w1m = np.array(W1, dtype=f32, copy=True)
    w1m[_NCLIN * _FV:, :] /= f32(_NPIX)
    # Pack to SBUF layout [p, (chunk, n)]: w1p[p, c*HID+n] = w1m[c*FV+p, n].
    w1m = np.ascontiguousarray(
        w1m.reshape(_NCHUNK, _FV, _HID).transpose(1, 0, 2).reshape(_FV, -1))
    aux = np.empty((_BC, _HID + 3), dtype=f32)
    aux[:, :_HID] = np.asarray(W2, f32).reshape(1, _HID)
    aux[:, _HID] = np.asarray(b_g, f32)
    aux[:, _HID + 1] = f32(np.asarray(b2, f32).reshape(-1)[0])
    aux[:, _HID + 2] = f32(0.0)
    rowaux = np.empty((1, _HID + _BC), dtype=f32)
    rowaux[0, :_HID] = np.asarray(b1, f32)
    rowaux[0, _HID:] = f32(1.0)
    return gw, w1m, aux, rowaux


def _xt_for_core(clinical, image, k):
    sl = slice(k * _BC, (k + 1) * _BC)
    xc = np.ascontiguousarray(clinical[sl].transpose(2, 1, 0)).reshape(_FV, _CCOLS)
    xp = np.ascontiguousarray(image[sl].transpose(2, 1, 0)).reshape(_FV, _PCOLS)
    return np.ascontiguousarray(np.concatenate([xc, xp], axis=1))


def kernel(**inputs):
    clinical = np.asarray(inputs["clinical_embeddings"], np.float32)
    image = np.asarray(inputs["image_embeddings"], np.float32)
    gw, w1m, aux, rowaux = _host_prep(
        inputs["W_self"], inputs["W_msg"], inputs["b_g"],
        inputs["W1"], inputs["b1"], inputs["W2"], inputs["b2"],
    )

    if "nc" not in _CACHE:
        _CACHE["nc"] = _build_bass()
    nc = _CACHE["nc"]

    in_maps = [
        {
            "xt": _xt_for_core(clinical, image, k),
            "w1": w1m,
            "gw": gw,
            "aux": aux,
            "rowaux": rowaux,
        }
        for k in range(_NCORES)
    ]

    from concourse.bass_utils import run_bass_kernel_spmd

    res = run_bass_kernel_spmd(
        nc, in_maps, core_ids=list(range(_NCORES)),
        trace=bool(_CACHE.get("trace", False)),
        **_CACHE.get("run_kwargs", {}),
    )
    _CACHE["last_results"] = res
    out = np.concatenate([r["out"] for r in res.results], axis=0)
    return np.ascontiguousarray(out.astype(np.float32))



# revision 2
# speedup vs baseline: 1.4409x; 1.4409x over previous
"""Trainium2 Bass kernel for nn_Network_63763084476816 (GNN message passing).

The batched graph is structurally fixed: per graph, 38 clinical + 36 pixel
nodes, self-edges everywhere, and a complete bipartite pixel<->clinical edge
set.  Mean aggregation therefore collapses to dense math:

    h_c = relu(x_c @ (W_self + W_msg/37) + S_pix @ (W_msg/37) + b_g)
    h_p = relu(x_p @ (W_self + W_msg/39) + S_clin @ (W_msg/39) + b_g)
    gap = mean_p h_p
    out = relu([h_c | gap] @ W1 + b1) @ W2 + b2

Sharding: pure data parallel, 128 graphs per core on 8 cores; weights
(including W1) replicated.  Embeddings ship feature-major ([FV, node*BC+b])
so every matmul operand has its contraction dim on partitions.

v2: the kernel is HBM-bound (W1 fp32 alone was 10.2 MB/core), so the two
big streams (xt, W1) and the h-layer weights ship as bf16 - 7.9 MB/core
total, about half the fp32 traffic.  bf16 also keeps the PE on the
1-cycle/row path at any N and halves LDWEIGHTS time.  Node sums use pure
contiguous bf16 tree-folds (2x DVE packing; the old strided reduces ran
4x slower than contiguous).  PSUM evictions split across engines:
clinical h on the scalar engine (activation: bias+relu), pixel h on the
vector engine (tensor_scalar: +b_g then max 0), both writing bf16.
Accumulation stays fp32 in PSUM everywhere; the small MLP tail
(W2/b1/b2) stays fp32/f32r.
"""

import sys

for _p in ("/opt/trn_rl_repo",):
    if _p not in sys.path:
        sys.path.insert(0, _p)

import numpy as np

_B = 1024
_NCORES = 8
_BC = _B // _NCORES  # 128 graphs per core
_NCLIN = 38
_NPIX = 36
_FV = 128
_HID = 512
_NCHUNK = 39  # K-chunks of 128 in the 4992-wide MLP contraction
# K-chunks per W1 DMA group; last group tiny so the MLP tail after the
# final W1 arrival is one matmul.
_W1GROUPS = [8, 8, 8, 8, 6, 1]
_CCOLS = _NCLIN * _BC  # 4864
_PCOLS = _NPIX * _BC  # 4608

_CACHE = {}


def _build_bass():
    import concourse.bacc as bacc
    import concourse.mybir as mybir
    import concourse.tile as tile

    f32 = mybir.dt.float32
    f32r = mybir.dt.float32r
    bf16 = mybir.dt.bfloat16
    relu = mybir.ActivationFunctionType.Relu
    add_op = mybir.AluOpType.add
    max_op = mybir.AluOpType.max

    nc = bacc.Bacc("TRN2", target_bir_lowering=False, debug=False,
                   num_devices=_NCORES)

    xt_d = nc.dram_tensor("xt", [_FV, _CCOLS + _PCOLS], bf16, kind="ExternalInput")
    # W1 host-packed in the SBUF layout: [p, (chunk, n)] - long contiguous
    # per-partition runs for every DMA.
    w1_d = nc.dram_tensor("w1", [_FV, _NCHUNK * _HID], bf16, kind="ExternalInput")
    gw_d = nc.dram_tensor("gw", [_FV, 4 * _FV], bf16, kind="ExternalInput")
    aux_d = nc.dram_tensor("aux", [_BC, _HID + 3], f32, kind="ExternalInput")
    rowaux_d = nc.dram_tensor("rowaux", [1, _HID + _BC], f32r, kind="ExternalInput")
    out_d = nc.dram_tensor("out", [_BC, 1], f32, kind="ExternalOutput")

    _LOWP = "bf16 operands/outputs; matmul accumulation stays fp32 in PSUM"

    with tile.TileContext(nc) as tc:
        with tc.tile_pool(name="main", bufs=1) as pool, \
             tc.tile_pool(name="hps", bufs=6, space="PSUM") as pps, \
             tc.tile_pool(name="zps", bufs=1, space="PSUM") as ppz:

            # Small parameter loads on the scalar (ACT) HWDGE ring so they
            # don't delay the big streams on the sync (SP) ring.
            gwsb = pool.tile([_FV, 4 * _FV], bf16, name="gwsb", tag="gwsb")
            nc.scalar.dma_start(gwsb[:], gw_d.ap())
            auxsb = pool.tile([_BC, _HID + 3], f32, name="auxsb", tag="auxsb")
            nc.scalar.dma_start(auxsb[:], aux_d.ap())
            rowsb = pool.tile([1, _HID + _BC], f32r, name="rowsb", tag="rowsb")
            nc.scalar.dma_start(rowsb[:], rowaux_d.ap())

            # Node embeddings, feature-major bf16.  Pixel section first (its
            # sum gates the clinical h blocks, which run first); both
            # sections in two parts so the S partial folds start before the
            # full section lands.  Clinical splits 20+18 blocks so each part
            # folds to an integral block count.
            xt = pool.tile([_FV, _CCOLS + _PCOLS], bf16, name="xt", tag="xt")
            _PH = _PCOLS // 2  # 2304 = 18 pixel blocks
            _CH = 20 * _BC  # 2560 = 20 clinical blocks
            nc.sync.dma_start(xt[:, _CCOLS:_CCOLS + _PH], xt_d.ap()[:, _CCOLS:_CCOLS + _PH])
            nc.sync.dma_start(xt[:, _CCOLS + _PH:], xt_d.ap()[:, _CCOLS + _PH:])
            nc.sync.dma_start(xt[:, :_CH], xt_d.ap()[:, :_CH])
            nc.sync.dma_start(xt[:, _CH:_CCOLS], xt_d.ap()[:, _CH:_CCOLS])

            # W1 streamed in groups; group g holds K-chunks as [FV, gch, HID].
            # W1 after xt on the same sync ring: FIFO order doubles as a
            # priority order, so the xt stream (which gates all compute)
            # never contends with the W1 stream.
            w1sb = []
            c0 = 0
            for g, gch in enumerate(_W1GROUPS):
                t = pool.tile([_FV, gch, _HID], bf16, name=f"w1sb{g}", tag=f"w1sb{g}")
                nc.sync.dma_start(
                    t[:],
                    w1_d.ap()[:, c0 * _HID:(c0 + gch) * _HID].rearrange(
                        "p (c n) -> p c n", c=gch),
                )
                w1sb.append(t)
                c0 += gch

            # ---- Per-graph node sums S[f, b] via contiguous bf16 tree-folds
            # (all on the vector engine; 16-bit packing gives 2 adds/cycle).
            # Replicated to 4 copies for the N=512 aggregate matmuls.

            # S_pix: 36 blocks, halves of 18 blocks fold independently.
            P0 = _CCOLS
            upx = pool.tile([_FV, 2304], bf16, name="upx", tag="upx")
            vpx = pool.tile([_FV, 1152], bf16, name="vpx", tag="vpx")
            wpx = pool.tile([_FV, 896], bf16, name="wpx", tag="wpx")
            s4pix = pool.tile([_FV, 4 * _BC], bf16, name="s4pix", tag="s4pix")
            nc.vector.tensor_add(upx[:, :1152], xt[:, P0:P0 + 1152],
                                 xt[:, P0 + 1152:P0 + 2304])
            nc.vector.tensor_add(upx[:, 1152:], xt[:, P0 + 2304:P0 + 3456],
                                 xt[:, P0 + 3456:P0 + 4608])
            nc.vector.tensor_add(vpx[:], upx[:, :1152], upx[:, 1152:])
            nc.vector.tensor_add(wpx[:, :512], vpx[:, :512], vpx[:, 512:1024])
            nc.vector.tensor_add(wpx[:, 512:768], wpx[:, :256], wpx[:, 256:512])
            nc.vector.tensor_add(wpx[:, 768:896], wpx[:, 512:640], wpx[:, 640:768])
            nc.vector.tensor_add(s4pix[:, :_BC], wpx[:, 768:896], vpx[:, 1024:1152])
            nc.vector.tensor_copy(s4pix[:, _BC:2 * _BC], s4pix[:, :_BC])
            nc.vector.tensor_copy(s4pix[:, 2 * _BC:], s4pix[:, :2 * _BC])

            # S_clin: parts of 20 + 18 blocks.
            ucl = pool.tile([_FV, 2432], bf16, name="ucl", tag="ucl")
            vcl = pool.tile([_FV, 1152], bf16, name="vcl", tag="vcl")
            wcl = pool.tile([_FV, 1024], bf16, name="wcl", tag="wcl")
            s4clin = pool.tile([_FV, 4 * _BC], bf16, name="s4clin", tag="s4clin")
            nc.vector.tensor_add(ucl[:, :1280], xt[:, :1280], xt[:, 1280:2560])
            nc.vector.tensor_add(ucl[:, 1280:], xt[:, 2560:3712], xt[:, 3712:4864])
            nc.vector.tensor_add(vcl[:], ucl[:, :1152], ucl[:, 1280:2432])
            nc.vector.tensor_add(wcl[:, :512], vcl[:, :512], vcl[:, 512:1024])
            nc.vector.tensor_add(wcl[:, 512:768], wcl[:, :256], wcl[:, 256:512])
            nc.vector.tensor_add(wcl[:, 768:896], wcl[:, 512:640], wcl[:, 640:768])
            nc.vector.tensor_add(wcl[:, 896:1024], wcl[:, 768:896], vcl[:, 1024:1152])
            nc.vector.tensor_add(s4clin[:, :_BC], wcl[:, 896:1024], ucl[:, 1152:1280])
            nc.vector.tensor_copy(s4clin[:, _BC:2 * _BC], s4clin[:, :_BC])
            nc.vector.tensor_copy(s4clin[:, 2 * _BC:], s4clin[:, :2 * _BC])

            combT = pool.tile([_FV, _NCHUNK * _BC], bf16, name="combT", tag="combT")
            hpT = pool.tile([_FV, _PCOLS], bf16, name="hpT", tag="hpT")
            bg_ap = auxsb[:, _HID:_HID + 1]

            def h_phase(nblk, a_ap, wm_ap, s4_ap, src0, dest, psname, evict):
                g0, gi = 0, 0
                while g0 < nblk:
                    gcnt = min(4, nblk - g0)
                    w = gcnt * _BC
                    ps = pps.tile([_FV, w], f32, name=f"{psname}{gi}", tag="hps")
                    nc.tensor.matmul(
                        ps[:], a_ap,
                        xt[:, src0 + g0 * _BC: src0 + (g0 + gcnt) * _BC],
                        start=True, stop=False,
                    )
                    nc.tensor.matmul(
                        ps[:], wm_ap, s4_ap[:, :w],
                        start=False, stop=True,
                    )
                    dst = dest[:, g0 * _BC: g0 * _BC + w]
                    with nc.allow_low_precision(reason=_LOWP):
                        if evict == "scalar":
                            nc.scalar.activation(dst, ps[:], relu, bias=bg_ap)
                        else:
                            # relu(ps + b_g): per-partition bias add, then
                            # clamp at 0 - one DVE pass, PSUM -> bf16 SBUF.
                            nc.vector.tensor_scalar(
                                out=dst, in0=ps[:], scalar1=bg_ap, scalar2=0.0,
                                op0=add_op, op1=max_op,
                            )
                    g0 += gcnt
                    gi += 1

            # h^T tiles: clinical into combT blocks 0..37 (scalar-engine
            # eviction), pixel into hpT (vector-engine eviction).
            h_phase(_NCLIN, gwsb[:, 0:_FV], gwsb[:, 2 * _FV:3 * _FV], s4pix,
                    0, combT, "psc", "scalar")
            h_phase(_NPIX, gwsb[:, _FV:2 * _FV], gwsb[:, 3 * _FV:4 * _FV], s4clin,
                    _CCOLS, hpT, "psp", "vector")

            # gap block (plain sum; the 1/36 is folded into W1's last rows).
            ugp = pool.tile([_FV, 2304], bf16, name="ugp", tag="ugp")
            vgp = pool.tile([_FV, 1152], bf16, name="vgp", tag="vgp")
            wgp = pool.tile([_FV, 896], bf16, name="wgp", tag="wgp")
            nc.vector.tensor_add(ugp[:, :1152], hpT[:, :1152], hpT[:, 1152:2304])
            nc.vector.tensor_add(ugp[:, 1152:], hpT[:, 2304:3456], hpT[:, 3456:4608])
            nc.vector.tensor_add(vgp[:], ugp[:, :1152], ugp[:, 1152:])
            nc.vector.tensor_add(wgp[:, :512], vgp[:, :512], vgp[:, 512:1024])
            nc.vector.tensor_add(wgp[:, 512:768], wgp[:, :256], wgp[:, 256:512])
            nc.vector.tensor_add(wgp[:, 768:896], wgp[:, 512:640], wgp[:, 640:768])
            nc.vector.tensor_add(combT[:, _NCLIN * _BC:], wgp[:, 768:896],
                                 vgp[:, 1024:1152])

            # MLP layer 1: psz[b, n] = sum_k combined[b, k] W1[k, n] (+ b1).
            # Emission order = PE FIFO order: early-arriving W1 groups first,
            # then the b1 matmul and the gap chunk (ready mid-stream), and the
            # last-arriving W1 groups at the end so nothing head-blocks.
            psz = ppz.tile([_BC, _HID], f32, name="psz", tag="psz")

            def mlp_chunk(k, start, stop):
                goff = 0
                for g, gch in enumerate(_W1GROUPS):
                    if k < goff + gch:
                        nc.tensor.matmul(
                            psz[:],
                            combT[:, k * _BC:(k + 1) * _BC],
                            w1sb[g][:, k - goff, :],
                            start=start, stop=stop,
                        )
                        return
                    goff += gch

            for k in range(32):  # groups 0-3 (chunks 0..31)
                mlp_chunk(k, start=(k == 0), stop=False)
            nc.tensor.matmul(psz[:], rowsb[:, _HID:], rowsb[:, :_HID],
                             start=False, stop=False)  # + b1
            for k in range(32, 38):  # group 4
                mlp_chunk(k, start=False, stop=False)
            # chunk 38 = gap x W1 group 5: both the gap h-values and the last
            # W1 bytes are the latest to arrive, so this goes last.
            mlp_chunk(38, start=False, stop=True)

            # MLP layer 2 fused: one DVE op does relu (max with 0), the W2
            # multiply, and the free-dim sum, reading psz directly from PSUM.
            # (tensor_tensor_reduce wedges the device on this path;
            # scalar_tensor_tensor with accum_out is HW-verified.)
            zw = pool.tile([_BC, _HID], f32, name="zw", tag="zw")
            osum = pool.tile([_BC, 1], f32, name="osum", tag="osum")
            nc.vector.scalar_tensor_tensor(
                out=zw[:], in0=psz[:], scalar=0.0, in1=auxsb[:, :_HID],
                op0=mybir.AluOpType.max, op1=mybir.AluOpType.mult,
                accum_out=osum[:],
            )
            ofin = pool.tile([_BC, 1], f32, name="ofin", tag="ofin")
            nc.vector.tensor_add(ofin[:], osum[:], auxsb[:, _HID + 1:_HID + 2])
            nc.sync.dma_start(out_d.ap(), ofin[:])

    nc.compile()
    return nc


def _host_prep(W_self, W_msg, b_g, W1, b1, W2, b2):
    import ml_dtypes

    f32 = np.float32
    bf16 = ml_dtypes.bfloat16
    wmc = np.asarray(W_msg, f32) / f32(37.0)
    wmp = np.asarray(W_msg, f32) / f32(39.0)
    ws = np.asarray(W_self, f32)
    gw = np.ascontiguousarray(
        np.hstack([ws + wmc, ws + wmp, wmc, wmp]).astype(bf16))
    w1m = np.array(W1, dtype=f32, copy=True)
    w1m[_NCLIN * _FV:, :] /= f32(_NPIX)
    # Pack to SBUF layout [p, (chunk, n)]: w1p[p, c*HID+n] = w1m[c*FV+p, n].
    w1m = np.ascontiguousarray(
        w1m.reshape(_NCHUNK, _FV, _HID).transpose(1, 0, 2).reshape(_FV, -1)
        .astype(bf16))
    aux = np.empty((_BC, _HID + 3), dtype=f32)
    aux[:, :_HID] = np.asarray(W2, f32).reshape(1, _HID)
    aux[:, _HID] = np.asarray(b_g, f32)
    aux[:, _HID + 1] = f32(np.asarray(b2, f32).reshape(-1)[0])
    aux[:, _HID + 2] = f32(0.0)
    rowaux = np.empty((1, _HID + _BC), dtype=f32)
    rowaux[0, :_HID] = np.asarray(b1, f32)
    rowaux[0, _HID:] = f32(1.0)
    return gw, w1m, aux, rowaux


def _xt_for_core(clinical, image, k):
    import ml_dtypes

    bf16 = ml_dtypes.bfloat16
    sl = slice(k * _BC, (k + 1) * _BC)
    xc = np.ascontiguousarray(clinical[sl].transpose(2, 1, 0)).reshape(_FV, _CCOLS)
    xp = np.ascontiguousarray(image[sl].transpose(2, 1, 0)).reshape(_FV, _PCOLS)
    return np.ascontiguousarray(
        np.concatenate([xc, xp], axis=1).astype(bf16))


def kernel(**inputs):
    clinical = np.asarray(inputs["clinical_embeddings"], np.float32)
    image = np.asarray(inputs["image_embeddings"], np.float32)
    gw, w1m, aux, rowaux = _host_prep(
        inputs["W_self"], inputs["W_msg"], inputs["b_g"],
        inputs["W1"], inputs["b1"], inputs["W2"], inputs["b2"],
    )

    if "nc" not in _CACHE:
        _CACHE["nc"] = _build_bass()
    nc = _CACHE["nc"]

    in_maps = [
        {
            "xt": _xt_for_core(clinical, image, k),
            "w1": w1m,
            "gw": gw,
            "aux": aux,
            "rowaux": rowaux,
        }
        for k in range(_NCORES)
    ]

    from concourse.bass_utils import run_bass_kernel_spmd

    res = run_bass_kernel_spmd(
        nc, in_maps, core_ids=list(range(_NCORES)),
        trace=bool(_CACHE.get("trace", False)),
        **_CACHE.get("run_kwargs", {}),
    )
    _CACHE["last_results"] = res
    out = np.concatenate([r["out"] for r in res.results], axis=0)
    return np.ascontiguousarray(out.astype(np.float32))


# revision 3
# speedup vs baseline: 1.5756x; 1.0935x over previous
"""Trainium2 Bass kernel for nn_Network_63763084476816 (GNN message passing).

The batched graph is structurally fixed: per graph, 38 clinical + 36 pixel
nodes, self-edges everywhere, and a complete bipartite pixel<->clinical edge
set.  Mean aggregation therefore collapses to dense math:

    h_c = relu(x_c @ (W_self + W_msg/37) + S_pix @ (W_msg/37) + b_g)
    h_p = relu(x_p @ (W_self + W_msg/39) + S_clin @ (W_msg/39) + b_g)
    gap = mean_p h_p
    out = relu([h_c | gap] @ W1 + b1) @ W2 + b2

Sharding: pure data parallel, 128 graphs per core on 8 cores; weights
(including W1) replicated.  Embeddings ship feature-major ([FV, node*BC+b])
so every matmul operand has its contraction dim on partitions.

The kernel is HBM-bound: the big streams (xt, W1) and the h-layer weights
ship as bf16 (7.9 MB/core, half the fp32 traffic); bf16 also keeps the PE
on the 1-cycle/row path at any N and halves LDWEIGHTS.  Node sums use
contiguous bf16 tree-folds on the DVE (2x packing; strided reduces run 4x
slower).  h PSUM tiles are 1024 cols (2 banks) so each eviction is one
wide instruction, and evictions alternate scalar (activation: bias+relu)
and vector (tensor_scalar: +b_g, max 0) so the two PSUM-capable engines
drain banks in parallel - eviction throughput, not the PE, paces the h
phase.  PSUM h tiles align exactly with the W1 DMA groups so each MLP
chunk's stationary block is evicted well before its W1 bytes land.  The
scalar result [BC,1] is PE-transposed to one partition before the store:
a [BC,1] DMA shatters into BC 4-byte descriptors whose completion
semaphore costs ~7us; the [1,BC] form is a single 512B descriptor.
Accumulation stays fp32 in PSUM everywhere; W2/b1/b2 stay fp32/f32r.
"""

import sys

for _p in ("/opt/trn_rl_repo",):
    if _p not in sys.path:
        sys.path.insert(0, _p)

import numpy as np

_B = 1024
_NCORES = 8
_BC = _B // _NCORES  # 128 graphs per core
_NCLIN = 38
_NPIX = 36
_FV = 128
_HID = 512
_NCHUNK = 39  # K-chunks of 128 in the 4992-wide MLP contraction
# K-chunks per W1 DMA group; groups match the h PSUM tiles (8 chunks =
# 1024 cols = one 2-bank tile); last group tiny so the MLP tail after the
# final W1 arrival is one matmul.
_W1GROUPS = [8, 8, 8, 8, 6, 1]
_CCOLS = _NCLIN * _BC  # 4864
_PCOLS = _NPIX * _BC  # 4608

_CACHE = {}


def _build_bass():
    import concourse.bacc as bacc
    import concourse.mybir as mybir
    import concourse.tile as tile

    f32 = mybir.dt.float32
    f32r = mybir.dt.float32r
    bf16 = mybir.dt.bfloat16
    relu = mybir.ActivationFunctionType.Relu
    add_op = mybir.AluOpType.add
    max_op = mybir.AluOpType.max

    nc = bacc.Bacc("TRN2", target_bir_lowering=False, debug=False,
                   num_devices=_NCORES)

    xt_d = nc.dram_tensor("xt", [_FV, _CCOLS + _PCOLS], bf16, kind="ExternalInput")
    # W1 host-packed in the SBUF layout: [p, (chunk, n)] - long contiguous
    # per-partition runs for every DMA.
    w1_d = nc.dram_tensor("w1", [_FV, _NCHUNK * _HID], bf16, kind="ExternalInput")
    gw_d = nc.dram_tensor("gw", [_FV, 4 * _FV], bf16, kind="ExternalInput")
    aux_d = nc.dram_tensor("aux", [_BC, _HID + 3], f32, kind="ExternalInput")
    rowaux_d = nc.dram_tensor("rowaux", [1, _HID + _BC], f32r, kind="ExternalInput")
    ident_d = nc.dram_tensor("ident", [_FV, _FV], f32, kind="ExternalInput")
    out_d = nc.dram_tensor("out", [1, _BC], f32, kind="ExternalOutput")

    _LOWP = "bf16 operands/outputs; matmul accumulation stays fp32 in PSUM"

    with tile.TileContext(nc) as tc:
        with tc.tile_pool(name="main", bufs=1) as pool, \
             tc.tile_pool(name="hps", bufs=3, space="PSUM") as pps, \
             tc.tile_pool(name="zps", bufs=1, space="PSUM") as ppz:

            # Small parameter loads on the scalar (ACT) HWDGE ring so they
            # don't delay the big streams on the sync (SP) ring.
            gwsb = pool.tile([_FV, 4 * _FV], bf16, name="gwsb", tag="gwsb")
            nc.scalar.dma_start(gwsb[:], gw_d.ap())
            auxsb = pool.tile([_BC, _HID + 3], f32, name="auxsb", tag="auxsb")
            nc.scalar.dma_start(auxsb[:], aux_d.ap())
            rowsb = pool.tile([1, _HID + _BC], f32r, name="rowsb", tag="rowsb")
            nc.scalar.dma_start(rowsb[:], rowaux_d.ap())
            idsb = pool.tile([_FV, _FV], f32, name="idsb", tag="idsb")
            nc.scalar.dma_start(idsb[:], ident_d.ap())

            # Node embeddings, feature-major bf16.  Pixel section first (its
            # sum gates the clinical h blocks, which run first); both
            # sections in two parts so the S partial folds start before the
            # full section lands.  Clinical splits 20+18 blocks so each part
            # folds to an integral block count.
            xt = pool.tile([_FV, _CCOLS + _PCOLS], bf16, name="xt", tag="xt")
            _PH = _PCOLS // 2  # 2304 = 18 pixel blocks
            _CH = 20 * _BC  # 2560 = 20 clinical blocks
            nc.sync.dma_start(xt[:, _CCOLS:_CCOLS + _PH], xt_d.ap()[:, _CCOLS:_CCOLS + _PH])
            nc.sync.dma_start(xt[:, _CCOLS + _PH:], xt_d.ap()[:, _CCOLS + _PH:])
            nc.sync.dma_start(xt[:, :_CH], xt_d.ap()[:, :_CH])
            nc.sync.dma_start(xt[:, _CH:_CCOLS], xt_d.ap()[:, _CH:_CCOLS])

            # W1 streamed in groups; group g holds K-chunks as [FV, gch, HID].
            # W1 after xt on the same sync ring: FIFO order doubles as a
            # priority order, so the xt stream (which gates all compute)
            # never contends with the W1 stream.
            w1sb = []
            c0 = 0
            for g, gch in enumerate(_W1GROUPS):
                t = pool.tile([_FV, gch, _HID], bf16, name=f"w1sb{g}", tag=f"w1sb{g}")
                nc.sync.dma_start(
                    t[:],
                    w1_d.ap()[:, c0 * _HID:(c0 + gch) * _HID].rearrange(
                        "p (c n) -> p c n", c=gch),
                )
                w1sb.append(t)
                c0 += gch

            # ---- Per-graph node sums S[f, b] via contiguous bf16 tree-folds
            # (vector engine; 16-bit packing gives 2 adds/cycle).  Replicated
            # to 4 copies for the N=512 aggregate matmuls.

            # S_pix: 36 blocks, halves of 18 blocks fold independently.
            P0 = _CCOLS
            upx = pool.tile([_FV, 2304], bf16, name="upx", tag="upx")
            vpx = pool.tile([_FV, 1152], bf16, name="vpx", tag="vpx")
            wpx = pool.tile([_FV, 896], bf16, name="wpx", tag="wpx")
            s4pix = pool.tile([_FV, 4 * _BC], bf16, name="s4pix", tag="s4pix")
            nc.vector.tensor_add(upx[:, :1152], xt[:, P0:P0 + 1152],
                                 xt[:, P0 + 1152:P0 + 2304])
            nc.vector.tensor_add(upx[:, 1152:], xt[:, P0 + 2304:P0 + 3456],
                                 xt[:, P0 + 3456:P0 + 4608])
            nc.vector.tensor_add(vpx[:], upx[:, :1152], upx[:, 1152:])
            nc.vector.tensor_add(wpx[:, :512], vpx[:, :512], vpx[:, 512:1024])
            nc.vector.tensor_add(wpx[:, 512:768], wpx[:, :256], wpx[:, 256:512])
            nc.vector.tensor_add(wpx[:, 768:896], wpx[:, 512:640], wpx[:, 640:768])
            nc.vector.tensor_add(s4pix[:, :_BC], wpx[:, 768:896], vpx[:, 1024:1152])
            nc.vector.tensor_copy(s4pix[:, _BC:2 * _BC], s4pix[:, :_BC])
            nc.vector.tensor_copy(s4pix[:, 2 * _BC:], s4pix[:, :2 * _BC])

            # S_clin first-part fold can run while part2 streams.
            ucl = pool.tile([_FV, 2432], bf16, name="ucl", tag="ucl")
            vcl = pool.tile([_FV, 1152], bf16, name="vcl", tag="vcl")
            wcl = pool.tile([_FV, 1024], bf16, name="wcl", tag="wcl")
            s4clin = pool.tile([_FV, 4 * _BC], bf16, name="s4clin", tag="s4clin")
            nc.vector.tensor_add(ucl[:, :1280], xt[:, :1280], xt[:, 1280:2560])

            def s4clin_tail():
                nc.vector.tensor_add(ucl[:, 1280:], xt[:, 2560:3712],
                                     xt[:, 3712:4864])
                nc.vector.tensor_add(vcl[:], ucl[:, :1152], ucl[:, 1280:2432])
                nc.vector.tensor_add(wcl[:, :512], vcl[:, :512], vcl[:, 512:1024])
                nc.vector.tensor_add(wcl[:, 512:768], wcl[:, :256], wcl[:, 256:512])
                nc.vector.tensor_add(wcl[:, 768:896], wcl[:, 512:640],
                                     wcl[:, 640:768])
                nc.vector.tensor_add(wcl[:, 896:1024], wcl[:, 768:896],
                                     vcl[:, 1024:1152])
                nc.vector.tensor_add(s4clin[:, :_BC], wcl[:, 896:1024],
                                     ucl[:, 1152:1280])
                nc.vector.tensor_copy(s4clin[:, _BC:2 * _BC], s4clin[:, :_BC])
                nc.vector.tensor_copy(s4clin[:, 2 * _BC:], s4clin[:, :2 * _BC])

            combT = pool.tile([_FV, _NCHUNK * _BC], bf16, name="combT", tag="combT")
            hpT = pool.tile([_FV, _PCOLS], bf16, name="hpT", tag="hpT")
            bg_ap = auxsb[:, _HID:_HID + 1]

            def evict(engine, dst, ps_ap):
                with nc.allow_low_precision(reason=_LOWP):
                    if engine == "s":
                        nc.scalar.activation(dst, ps_ap, relu, bias=bg_ap)
                    else:
                        # relu(ps + b_g): per-partition bias add then clamp
                        # at 0 - one DVE pass, PSUM -> bf16 SBUF.
                        nc.vector.tensor_scalar(
                            out=dst, in0=ps_ap, scalar1=bg_ap, scalar2=0.0,
                            op0=add_op, op1=max_op,
                        )

            def h_tile(width, a_ap, wm_ap, s4_ap, src0, dest, d0, name, eng,
                       pre_evict=None):
                # One 2-bank PSUM tile: up-to-512-col matmul pairs, then a
                # single wide eviction on the assigned engine.
                ps = pps.tile([_FV, width], f32, name=name, tag="hps")
                o = 0
                while o < width:
                    w = min(512, width - o)
                    nc.tensor.matmul(
                        ps[:, o:o + w], a_ap,
                        xt[:, src0 + d0 + o: src0 + d0 + o + w],
                        start=True, stop=False,
                    )
                    nc.tensor.matmul(
                        ps[:, o:o + w], wm_ap, s4_ap[:, :w],
                        start=False, stop=True,
                    )
                    o += w
                if pre_evict is not None:
                    pre_evict()  # vector-FIFO work that must precede this evict
                evict(eng, dest[:, d0:d0 + width], ps[:])

            # Clinical h -> combT blocks 0..37.  Tile widths match the W1
            # groups (8 chunks = 1024).  Eviction engines alternate so
            # scalar and vector drain PSUM concurrently; the S_clin fold
            # tail is spliced into the vector FIFO before the first
            # vector-owned eviction (both are ready around the same time,
            # and the fold gates the pixel s4 matmuls).
            a_c = gwsb[:, 0:_FV]
            wm_c = gwsb[:, 2 * _FV:3 * _FV]
            h_tile(1024, a_c, wm_c, s4pix, 0, combT, 0, "psc0", "s")
            h_tile(1024, a_c, wm_c, s4pix, 0, combT, 1024, "psc1", "v",
                   pre_evict=s4clin_tail)
            h_tile(1024, a_c, wm_c, s4pix, 0, combT, 2048, "psc2", "s")
            h_tile(1024, a_c, wm_c, s4pix, 0, combT, 3072, "psc3", "v")
            h_tile(768, a_c, wm_c, s4pix, 0, combT, 4096, "psc4", "s")

            # Pixel h -> hpT.
            a_p = gwsb[:, _FV:2 * _FV]
            wm_p = gwsb[:, 3 * _FV:4 * _FV]
            h_tile(1024, a_p, wm_p, s4clin, _CCOLS, hpT, 0, "psp0", "v")
            h_tile(1024, a_p, wm_p, s4clin, _CCOLS, hpT, 1024, "psp1", "s")
            h_tile(1024, a_p, wm_p, s4clin, _CCOLS, hpT, 2048, "psp2", "v")
            h_tile(1024, a_p, wm_p, s4clin, _CCOLS, hpT, 3072, "psp3", "s")
            h_tile(512, a_p, wm_p, s4clin, _CCOLS, hpT, 4096, "psp4", "v")

            # gap block (plain sum; the 1/36 is folded into W1's last rows).
            ugp = pool.tile([_FV, 2304], bf16, name="ugp", tag="ugp")
            vgp = pool.tile([_FV, 1152], bf16, name="vgp", tag="vgp")
            wgp = pool.tile([_FV, 896], bf16, name="wgp", tag="wgp")
            nc.vector.tensor_add(ugp[:, :1152], hpT[:, :1152], hpT[:, 1152:2304])
            nc.vector.tensor_add(ugp[:, 1152:], hpT[:, 2304:3456], hpT[:, 3456:4608])
            nc.vector.tensor_add(vgp[:], ugp[:, :1152], ugp[:, 1152:])
            nc.vector.tensor_add(wgp[:, :512], vgp[:, :512], vgp[:, 512:1024])
            nc.vector.tensor_add(wgp[:, 512:768], wgp[:, :256], wgp[:, 256:512])
            nc.vector.tensor_add(wgp[:, 768:896], wgp[:, 512:640], wgp[:, 640:768])
            nc.vector.tensor_add(combT[:, _NCLIN * _BC:], wgp[:, 768:896],
                                 vgp[:, 1024:1152])

            # MLP layer 1: psz[b, n] = sum_k combined[b, k] W1[k, n] (+ b1).
            # Emission order = PE FIFO order: early-arriving W1 groups first,
            # then the b1 matmul and the gap chunk (ready mid-stream), and the
            # last-arriving W1 groups at the end so nothing head-blocks.
            psz = ppz.tile([_BC, _HID], f32, name="psz", tag="psz")

            def mlp_chunk(k, start, stop):
                goff = 0
                for g, gch in enumerate(_W1GROUPS):
                    if k < goff + gch:
                        nc.tensor.matmul(
                            psz[:],
                            combT[:, k * _BC:(k + 1) * _BC],
                            w1sb[g][:, k - goff, :],
                            start=start, stop=stop,
                        )
                        return
                    goff += gch

            for k in range(32):  # groups 0-3 (chunks 0..31)
                mlp_chunk(k, start=(k == 0), stop=False)
            nc.tensor.matmul(psz[:], rowsb[:, _HID:], rowsb[:, :_HID],
                             start=False, stop=False)  # + b1
            for k in range(32, 38):  # group 4
                mlp_chunk(k, start=False, stop=False)
            # chunk 38 = gap x W1 group 5: both the gap h-values and the last
            # W1 bytes are the latest to arrive, so this goes last.
            mlp_chunk(38, start=False, stop=True)

            # MLP layer 2 fused: one DVE op does relu (max with 0), the W2
            # multiply, and the free-dim sum, reading psz directly from PSUM.
            # (tensor_tensor_reduce wedges the device on this path;
            # scalar_tensor_tensor with accum_out is HW-verified.)
            zw = pool.tile([_BC, _HID], f32, name="zw", tag="zw")
            osum = pool.tile([_BC, 1], f32, name="osum", tag="osum")
            nc.vector.scalar_tensor_tensor(
                out=zw[:], in0=psz[:], scalar=0.0, in1=auxsb[:, :_HID],
                op0=mybir.AluOpType.max, op1=mybir.AluOpType.mult,
                accum_out=osum[:],
            )
            ofin = pool.tile([_BC, 1], f32, name="ofin", tag="ofin")
            nc.vector.tensor_add(ofin[:], osum[:], auxsb[:, _HID + 1:_HID + 2])
            # Gather the per-partition scalars onto one partition (PE
            # transpose) so the output store is one contiguous descriptor.
            pst = ppz.tile([1, _BC], f32, name="pst", tag="pst")
            nc.tensor.transpose(pst[:], ofin[:], idsb[:])
            orow = pool.tile([1, _BC], f32, name="orow", tag="orow")
            nc.vector.tensor_copy(orow[:], pst[:])
            nc.sync.dma_start(out_d.ap(), orow[:])

    nc.compile()
    return nc


def _host_prep(W_self, W_msg, b_g, W1, b1, W2, b2):
    import ml_dtypes

    f32 = np.float32
    bf16 = ml_dtypes.bfloat16
    wmc = np.asarray(W_msg, f32) / f32(37.0)
    wmp = np.asarray(W_msg, f32) / f32(39.0)
    ws = np.asarray(W_self, f32)
    gw = np.ascontiguousarray(
        np.hstack([ws + wmc, ws + wmp, wmc, wmp]).astype(bf16))
    w1m = np.array(W1, dtype=f32, copy=True)
    w1m[_NCLIN * _FV:, :] /= f32(_NPIX)
    # Pack to SBUF layout [p, (chunk, n)]: w1p[p, c*HID+n] = w1m[c*FV+p, n].
    w1m = np.ascontiguousarray(
        w1m.reshape(_NCHUNK, _FV, _HID).transpose(1, 0, 2).reshape(_FV, -1)
        .astype(bf16))
    aux = np.empty((_BC, _HID + 3), dtype=f32)
    aux[:, :_HID] = np.asarray(W2, f32).reshape(1, _HID)
    aux[:, _HID] = np.asarray(b_g, f32)
    aux[:, _HID + 1] = f32(np.asarray(b2, f32).reshape(-1)[0])
    aux[:, _HID + 2] = f32(0.0)
    rowaux = np.empty((1, _HID + _BC), dtype=f32)
    rowaux[0, :_HID] = np.asarray(b1, f32)
    rowaux[0, _HID:] = f32(1.0)
    ident = np.eye(_FV, dtype=f32)
    return gw, w1m, aux, rowaux, ident


def _xt_for_core(clinical, image, k):
    import ml_dtypes

    bf16 = ml_dtypes.bfloat16
    sl = slice(k * _BC, (k + 1) * _BC)
    xc = np.ascontiguousarray(clinical[sl].transpose(2, 1, 0)).reshape(_FV, _CCOLS)
    xp = np.ascontiguousarray(image[sl].transpose(2, 1, 0)).reshape(_FV, _PCOLS)
    return np.ascontiguousarray(
        np.concatenate([xc, xp], axis=1).astype(bf16))


def kernel(**inputs):
    clinical = np.asarray(inputs["clinical_embeddings"], np.float32)
    image = np.asarray(inputs["image_embeddings"], np.float32)
    gw, w1m, aux, rowaux, ident = _host_prep(
        inputs["W_self"], inputs["W_msg"], inputs["b_g"],
        inputs["W1"], inputs["b1"], inputs["W2"], inputs["b2"],
    )

    if "nc" not in _CACHE:
        _CACHE["nc"] = _build_bass()
    nc = _CACHE["nc"]

    in_maps = [
        {
            "xt": _xt_for_core(clinical, image, k),
            "w1": w1m,
            "gw": gw,
            "aux": aux,
            "rowaux": rowaux,
            "ident": ident,
        }
        for k in range(_NCORES)
    ]

    from concourse.bass_utils import run_bass_kernel_spmd

    res = run_bass_kernel_spmd(
        nc, in_maps, core_ids=list(range(_NCORES)),
        trace=bool(_CACHE.get("trace", False)),
        **_CACHE.get("run_kwargs", {}),
    )
    _CACHE["last_results"] = res
    out = np.concatenate(
        [r["out"].reshape(_BC, 1) for r in res.results], axis=0)
    return np.ascontiguousarray(out.astype(np.float32))
